# revision 13
# baseline (speedup 1.0000x reference)
"""Trainium2 Bass kernel for nn_BertSelfOutput (BiT 8-bit quantized BertSelfOutput).

Computation (see reference):
    wq = sym_quant(weight, clip=2.5, bits=8)       # layerwise scale s_w = 127/max|clip(w)|
    xq = sym_quant(hidden_states, clip=2.5, bits=8)
    h  = xq @ wq.T + bias
    y  = LayerNorm(h + input_tensor) * gamma + beta

Sharding: data-parallel over batch (8 cores, 1 batch element each); weight/bias/LN
params replicated.  Host-side marshalling is pure relayout (transpose/reshape): x,
res and the weight are laid out so every DMA is contiguous per SBUF partition
(8KB+ descriptors, near line rate), with the contraction dim on partitions.

Device algorithm per core (streaming, DMA/PE co-limited):
  - s_x is a compile-time constant 127/2.5: the layerwise clip at 2.5 makes
    max|clip(x)| == 2.5 whenever any element of the 2M-sample N(0,1) shard clips,
    which is a certainty at this size (kernel() enforces it with an exact host-side
    prescale fallback for the impossible case).  This removes the global abs-max
    barrier, so x streams: DMA block -> ACT scale+round to i16 -> DVE clamp to
    integer-valued bf16/fp8 -> matmul, tile by tile.
  - s_w is computed on device: per-chunk DVE abs-max rides each w DMA piece,
    gpsimd partition_all_reduce folds partitions, reciprocal.  Quantization rounds
    via the f32->i16 convert (nearest-even, matches jnp.round); the +-127
    tensor_scalar clamp realizes the clip exactly.
  - integer matmul on the PE; fp32 PSUM accumulation is exact (|sum| < 2^24).
    The bias rides in as a K=1 *bf16* matmul (bias*s_x*s_w in bf16: ~0.4% of a term
    that is ~2% of y's rms -- negligible), so accumulation groups never see fp32.
  - FP8_CHUNKS of the 8 k-chunks are stored as fp8e4m3 and contracted with
    DoubleRow matmuls (2 MACs/cell/cycle, K=256/instruction).  e4m3 keeps only 4
    significant bits of the 8-bit integers, adding ~2.6% rms noise to those
    products; the measured end-to-end max rel err is ~2.1e-2 for all 8 chunks and
    scales as sqrt(FP8_CHUNKS/8) (~1.5e-2 at 4) vs the 2e-2 gate.
  - LayerNorm is scale-invariant, so PSUM integers are never dequantized: the
    residual is scaled by s_x*s_w inside the fused scalar_tensor_tensor epilogue
    (which also emits the row sum), ACT Square+accum gives the sum of squares,
    per-2-tile stats -> rstd, DVE tensor_scalar applies (y-mu)*rstd.
  - output is stored bf16 tile-blocked (LN output is O(1); bf16 rounding is ~1e-3
    abs), widened to f32 and unblocked on the host (exact relayout).
"""

import numpy as np

P = 128
T = 2048   # tokens per core (S of one batch element)
H = 1024   # hidden
KO = 8     # k chunks of 128 (H / P)
XBLK = 2   # t-tiles per x/res/out DMA block
GROUP = 2  # t-tiles per stats group

S_X = 127.0 / 2.5  # layerwise activation quant scale (see module docstring)

# fp8 DoubleRow measured: bass/walrus lowers one DR matmul into TWO HW matmuls
# (518ns per K=256 vs bf16's 470ns) -- no win, plus ~1.5e-2 error.  Keep 0.
FP8_CHUNKS = 0  # how many of the KO k-chunks use fp8 DoubleRow (even; 0 disables)

_CACHE = {}


def _build(trivial_affine: bool, fp8_chunks: int, t=T, h=H):
    import concourse.bass as bass
    import concourse.bass_isa as bass_isa
    import concourse.bacc as bacc
    import concourse.mybir as mybir
    import concourse.tile as tile

    ko = h // P
    f8 = fp8_chunks
    assert f8 % 2 == 0 and 0 <= f8 <= ko
    nt = t // P                  # t-tiles
    nb = nt // XBLK             # x/res/out DMA blocks
    tb = XBLK * P               # tokens per block
    group = min(GROUP, nt)
    f32 = mybir.dt.float32
    bf16 = mybir.dt.bfloat16
    i16 = mybir.dt.int16
    fp8 = mybir.dt.float8e4
    Alu = mybir.AluOpType
    Act = mybir.ActivationFunctionType
    DR = mybir.MatmulPerfMode.DoubleRow

    nc = bacc.Bacc("TRN2", target_bir_lowering=False, debug=False)

    # x: [nb, P, ko*tb] f32, tile-contiguous: x4[b, p, c*tb + i] = x.T[c*P+p, b*tb + i]
    x4 = nc.dram_tensor("x4", [nb, P, ko * tb], f32, kind="ExternalInput").ap()
    # res: [nb, P, tb/P*h] f32 tile-blocked: res_m[b, p, q*h + o] = res[b*tb + q*P + p, o]
    res = nc.dram_tensor("res", [nb, P, XBLK * h], f32, kind="ExternalInput").ap()
    # w: [P, ko*h] f32: w3[p, c*h + o] = weight[o, c*P+p]
    w3 = nc.dram_tensor("w3", [P, ko * h], f32, kind="ExternalInput").ap()
    bias_d = nc.dram_tensor("bias", [h], f32, kind="ExternalInput").ap()
    gamma_d = nc.dram_tensor("gamma", [h], f32, kind="ExternalInput").ap()
    beta_d = nc.dram_tensor("beta", [h], f32, kind="ExternalInput").ap()
    # out: [nb, P, tb/P*h] bf16 tile-blocked (host unblocks + widens)
    out_d = nc.dram_tensor("out", [nb, P, XBLK * h], bf16, kind="ExternalOutput").ap()

    with tile.TileContext(nc) as tc:
        keep = tc.alloc_tile_pool(name="keep", bufs=1)
        p1 = tc.alloc_tile_pool(name="p1", bufs=1)

        # ---- persistent tiles ----
        wq8 = keep.tile([P, f8 * h], fp8, name="wq8") if f8 else None
        wqb = keep.tile([P, (ko - f8) * h], bf16, name="wqb") if f8 < ko else None
        ones_bf = keep.tile([1, P], bf16)
        nc.vector.memset(ones_bf, 1.0)
        bias_sb = keep.tile([1, h], f32)
        nc.sync.dma_start(out=bias_sb, in_=bias_d[None, :])
        bias_bf = keep.tile([1, h], bf16)  # bias * s_x * s_w
        wmax8 = keep.tile([P, ko], f32)
        wmax_p = keep.tile([P, 1], f32)
        wmax_a = keep.tile([P, 1], f32)    # all-reduced |w| max (same value on all partitions)
        s_w = keep.tile([P, 1], f32)
        ssw = keep.tile([P, 1], f32)       # s_x * s_w (residual/bias pre-scale)
        stat_sum = keep.tile([P, nt], f32)
        stat_sq = keep.tile([P, nt], f32)
        mu = keep.tile([P, nt], f32)
        rstd = keep.tile([P, nt], f32)
        nmurs = keep.tile([P, nt], f32)    # -mu * rstd
        if not trivial_affine:
            gam_rep = keep.tile([P, h], f32)
            bet_rep = keep.tile([P, h], f32)
            nc.sync.dma_start(out=gam_rep, in_=gamma_d[None, :].to_broadcast((P, h)))
            nc.sync.dma_start(out=bet_rep, in_=beta_d[None, :].to_broadcast((P, h)))

        # ---- streaming pools (x chain 0 interleaves into the w load below) ----
        pool_xf = tc.alloc_tile_pool(name="xf", bufs=3)
        pool_xi = tc.alloc_tile_pool(name="xi", bufs=2)
        pool_xq = tc.alloc_tile_pool(name="xq", bufs=4)
        pool_rt = tc.alloc_tile_pool(name="rt", bufs=3)
        pool_yt = tc.alloc_tile_pool(name="yt", bufs=6)
        pool_sq = tc.alloc_tile_pool(name="sq", bufs=2)
        pool_ot = tc.alloc_tile_pool(name="ot", bufs=2)
        pool_ps = tc.alloc_tile_pool(name="ps", bufs=4, space="PSUM")

        xqs = {}

        def emit_xchain(b):
            xf = pool_xf.tile([P, ko * tb], f32, tag="xf", name=f"xf_{b}")
            nc.sync.dma_start(out=xf, in_=x4[b])
            xi = pool_xi.tile([P, ko * tb], i16, tag="xi", name=f"xi_{b}")
            nc.scalar.activation(out=xi, in_=xf, func=Act.Identity, scale=S_X, bias=0.0)
            x8 = xb = None
            if f8:
                x8 = pool_xq.tile([P, f8 * tb], fp8, tag="x8", name=f"x8_{b}")
                nc.vector.tensor_scalar(
                    out=x8, in0=xi[:, : f8 * tb], scalar1=127.0, scalar2=-127.0,
                    op0=Alu.min, op1=Alu.max,
                )
            if f8 < ko:
                xb = pool_xq.tile([P, (ko - f8) * tb], bf16, tag="xb", name=f"xb_{b}")
                nc.vector.tensor_scalar(
                    out=xb, in0=xi[:, f8 * tb :], scalar1=127.0, scalar2=-127.0,
                    op0=Alu.min, op1=Alu.max,
                )
            xqs[b] = (x8, xb)

        # ---- load weight (gates the PE pipeline; x block 0 rides between w
        # chunks on the FIFO load ring so neither serializes the other fully) ----
        wf = p1.tile([P, ko * h], f32)
        for c2 in range(4):
            sl = slice(c2 * 2 * h, (c2 + 1) * 2 * h)
            nc.sync.dma_start(out=wf[:, sl], in_=w3[:, sl])
            for k in range(2):
                c = 2 * c2 + k
                nc.vector.tensor_reduce(
                    out=wmax8[:, c : c + 1], in_=wf[:, c * h : (c + 1) * h],
                    axis=mybir.AxisListType.X, op=Alu.max, apply_absolute_value=True,
                )
            if c2 == 1:
                emit_xchain(0)
        nc.vector.tensor_reduce(
            out=wmax_p, in_=wmax8, axis=mybir.AxisListType.X, op=Alu.max,
        )
        nc.gpsimd.partition_all_reduce(
            wmax_a, wmax_p, channels=P, reduce_op=bass_isa.ReduceOp.absmax,
        )
        # m = min(max|w|, clip); the +-127 clamp after rounding realizes the clip
        nc.vector.tensor_scalar_min(out=wmax_a, in0=wmax_a, scalar1=2.5)
        nc.vector.reciprocal(out=s_w, in_=wmax_a)
        nc.vector.tensor_scalar_mul(out=s_w, in0=s_w, scalar1=127.0)
        nc.vector.tensor_scalar_mul(out=ssw, in0=s_w, scalar1=S_X)
        nc.vector.tensor_scalar_mul(out=bias_bf, in0=bias_sb, scalar1=ssw[0:1, 0:1])

        # quantize weight: round(w*s_w) clamp [-127,127].  The HW f32->i16 convert
        # rounds nearest-even (matches jnp.round); min/max apply the clip during
        # the i16 -> bf16/fp8 convert (the clamped integers are bf16-exact).
        def wq_dst(c):
            if c < f8:
                return wq8[:, c * h : (c + 1) * h]
            return wqb[:, (c - f8) * h : (c - f8 + 1) * h]

        for c2 in range(4):
            wi16 = p1.tile([P, 2 * h], i16, tag="wi16", name=f"wi16_{c2}", bufs=2)
            nc.scalar.activation(
                out=wi16, in_=wf[:, c2 * 2 * h : (c2 + 1) * 2 * h],
                func=Act.Identity, scale=s_w, bias=0.0,
            )
            for k in range(2):
                c = 2 * c2 + k
                nc.vector.tensor_scalar(
                    out=wq_dst(c), in0=wi16[:, k * h : (k + 1) * h],
                    scalar1=127.0, scalar2=-127.0, op0=Alu.min, op1=Alu.max,
                )
        wq8_v = wq8.rearrange("p (c k o) -> p c k o", c=max(f8 // 2, 1), k=2) if f8 else None

        # ---- streaming main loop ----
        half = h // 2
        emit_xchain(1)
        yts = {}
        for b in range(nb):
            x8, xb = xqs.pop(b)
            x8_v = x8.rearrange("p (c k t) -> p c k t", c=max(f8 // 2, 1), k=2) if f8 else None
            rt = pool_rt.tile([P, XBLK * h], f32, tag="rt", name=f"rt_{b}")
            nc.sync.dma_start(out=rt, in_=res[b])

            for q in range(XBLK):
                j = XBLK * b + q
                ps = pool_ps.tile([P, h], f32, tag="ps", name=f"ps_{j}")
                for nf in range(2):
                    ocol = slice(nf * half, (nf + 1) * half)
                    nc.tensor.matmul(
                        ps[:, ocol], lhsT=ones_bf, rhs=bias_bf[:, ocol],
                        start=True, stop=False,
                    )
                for c2 in range(f8 // 2):
                    lhs = x8_v[:, c2, :, q * P : (q + 1) * P]
                    last = f8 == ko and c2 == f8 // 2 - 1
                    for nf in range(2):
                        ocol = slice(nf * half, (nf + 1) * half)
                        nc.tensor.matmul(
                            ps[:, ocol], lhsT=lhs, rhs=wq8_v[:, c2, :, ocol],
                            start=False, stop=last, perf_mode=DR,
                        )
                for ci in range(ko - f8):
                    lhs = xb[:, ci * tb + q * P : ci * tb + (q + 1) * P]
                    for nf in range(2):
                        ocol = slice(nf * half, (nf + 1) * half)
                        nc.tensor.matmul(
                            ps[:, ocol], lhsT=lhs,
                            rhs=wqb[:, ci * h + nf * half : ci * h + (nf + 1) * half],
                            start=False, stop=(ci == ko - f8 - 1),
                        )
                # y = res*(s_x*s_w) + psum ; accum_out = row-sum of y
                yt = pool_yt.tile([P, h], f32, tag="yt", name=f"yt_{j}")
                yts[j] = yt
                nc.vector.scalar_tensor_tensor(
                    out=yt, in0=rt[:, q * h : (q + 1) * h], scalar=ssw, in1=ps,
                    op0=Alu.mult, op1=Alu.add,
                    accum_out=stat_sum[:, j : j + 1],
                )
                # sum of squares on ACT (output tensor is a throwaway)
                sq = pool_sq.tile([P, h], bf16, tag="sq", name=f"sq_{j}")
                nc.scalar.activation(
                    out=sq, in_=yt, func=Act.Square,
                    accum_out=stat_sq[:, j : j + 1],
                )

            if b + 2 < nb:
                emit_xchain(b + 2)

            if (b + 1) * XBLK % group == 0:
                # ---- batched stats for the group ----
                g0 = (b + 1) * XBLK - group
                gsl = slice(g0, g0 + group)
                musl = mu[:, gsl]
                nc.vector.tensor_scalar_mul(out=musl, in0=stat_sum[:, gsl], scalar1=1.0 / h)
                var = rstd[:, gsl]  # slot reused: var -> sd -> rstd
                nc.vector.tensor_scalar_mul(out=var, in0=stat_sq[:, gsl], scalar1=1.0 / h)
                mu2 = pool_sq.tile([P, group], f32, tag="mu2", name=f"mu2_{g0}")
                nc.vector.tensor_tensor(mu2, musl, musl, Alu.mult)
                nc.vector.tensor_tensor(var, var, mu2, Alu.subtract)
                nc.scalar.sqrt(out=var, in_=var)
                nc.vector.reciprocal(out=var, in_=var)
                nc.vector.tensor_tensor(nmurs[:, gsl], musl, var, Alu.mult)
                nc.vector.tensor_scalar_mul(out=nmurs[:, gsl], in0=nmurs[:, gsl], scalar1=-1.0)
                # ---- normalize + store (per DMA block; stores ride the scalar
                # HWDGE ring so they never stall the x/res load ring) ----
                for b2 in range(g0 // XBLK, (g0 + group) // XBLK):
                    ot = pool_ot.tile([P, XBLK * h], bf16, tag="ot", name=f"ot_{b2}")
                    for q in range(XBLK):
                        j = XBLK * b2 + q
                        yt = yts.pop(j)
                        osl = slice(q * h, (q + 1) * h)
                        # (y - mu) * rstd on DVE (2x single-tensor mode), bf16 out
                        nc.vector.tensor_scalar(
                            out=ot[:, osl], in0=yt,
                            scalar1=rstd[:, j : j + 1], scalar2=nmurs[:, j : j + 1],
                            op0=Alu.mult, op1=Alu.add,
                        )
                        if not trivial_affine:
                            nc.vector.tensor_tensor(ot[:, osl], ot[:, osl], gam_rep, Alu.mult)
                            nc.vector.tensor_tensor(ot[:, osl], ot[:, osl], bet_rep, Alu.add)
                    nc.scalar.dma_start(out=out_d[b2], in_=ot)

        for p in (pool_ps, pool_ot, pool_sq, pool_yt, pool_rt, pool_xq, pool_xi, pool_xf, p1, keep):
            p.release()

    if not nc.is_finalized():
        nc.finalize()
    return nc


def _get_nc(trivial_affine: bool, t=T, h=H):
    key = (trivial_affine, FP8_CHUNKS, t, h)
    if key not in _CACHE:
        _CACHE[key] = _build(trivial_affine, FP8_CHUNKS, t, h)
    return _CACHE[key]


def make_in_maps(hidden_states, input_tensor, weight, bias, gamma, beta):
    """Host-side marshalling: pure relayout except the (practically impossible)
    no-clip fallback, where an exact prescale keeps the device math identical to
    the reference (see module docstring)."""
    hidden_states = np.asarray(hidden_states, dtype=np.float32)
    input_tensor = np.asarray(input_tensor, dtype=np.float32)
    weight = np.asarray(weight, dtype=np.float32)
    bias = np.asarray(bias, dtype=np.float32)
    gamma = np.asarray(gamma, dtype=np.float32)
    beta = np.asarray(beta, dtype=np.float32)

    B, S, HH = hidden_states.shape
    ko = HH // P
    nb = S // P // XBLK
    tb = XBLK * P

    m = float(np.abs(hidden_states).max())
    if m < 2.5:
        # reference scale would be 127/m; prescaling x/res/bias by 2.5/m makes
        # round(x'*S_X) the exact reference integers and S_X*res' the exact
        # reference residual scaling (LN is scale-invariant).
        f = 2.5 / m
        hidden_states = hidden_states * f
        input_tensor = input_tensor * f
        bias = bias * f

    w3 = np.ascontiguousarray(
        weight.T.reshape(ko, P, HH).transpose(1, 0, 2)
    ).reshape(P, ko * HH)
    in_maps = []
    for c in range(B):
        x4 = np.ascontiguousarray(
            hidden_states[c].reshape(nb, tb, ko, P).transpose(0, 3, 2, 1)
        ).reshape(nb, P, ko * tb)
        res_m = np.ascontiguousarray(
            input_tensor[c].reshape(nb, XBLK, P, HH).transpose(0, 2, 1, 3)
        ).reshape(nb, P, XBLK * HH)
        in_maps.append(
            {
                "x4": x4,
                "res": res_m,
                "w3": w3,
                "bias": bias,
                "gamma": gamma,
                "beta": beta,
            }
        )
    return in_maps


def gather_out(results, B, S=T, HH=H):
    """Unblock the tiled bf16 output and widen to f32 (exact relayout)."""
    nb = S // P // XBLK
    outs = []
    for c in range(B):
        o = np.asarray(results[c]["out"]).astype(np.float32)
        outs.append(o.reshape(nb, P, XBLK, HH).transpose(0, 2, 1, 3).reshape(S, HH))
    return np.stack(outs)


def kernel(hidden_states, input_tensor, weight, bias, gamma, beta):
    from concourse.bass_utils import run_bass_kernel_spmd

    gamma = np.asarray(gamma, dtype=np.float32)
    beta = np.asarray(beta, dtype=np.float32)
    B, S, HH = np.asarray(hidden_states).shape
    trivial = bool(np.all(gamma == 1.0) and np.all(beta == 0.0))
    nc = _get_nc(trivial, S, HH)

    in_maps = make_in_maps(hidden_states, input_tensor, weight, bias, gamma, beta)
    r = run_bass_kernel_spmd(nc, in_maps, core_ids=list(range(B)))
    return gather_out(r.results, B, S, HH)


# revision 16
# speedup vs baseline: 1.1986x; 1.1986x over previous
"""Trainium2 Bass kernel for nn_BertSelfOutput (BiT 8-bit quantized BertSelfOutput).

Computation (see reference):
    wq = sym_quant(weight, clip=2.5, bits=8)       # layerwise scale s_w = 127/max|clip(w)|
    xq = sym_quant(hidden_states, clip=2.5, bits=8)
    h  = xq @ wq.T + bias
    y  = LayerNorm(h + input_tensor) * gamma + beta

Sharding: data-parallel over batch (8 cores, 1 batch element each); weight/bias/LN
params replicated.  Host-side marshalling is pure relayout (transpose/reshape): x,
res and the weight are laid out so every DMA is contiguous per SBUF partition
(8KB+ descriptors, near line rate), with the contraction dim on partitions.

Device algorithm per core (streaming, DMA/PE co-limited):
  - s_x is a compile-time constant 127/2.5: the layerwise clip at 2.5 makes
    max|clip(x)| == 2.5 whenever any element of the 2M-sample N(0,1) shard clips,
    which is a certainty at this size (kernel() enforces it with an exact host-side
    prescale fallback for the impossible case).  This removes the global abs-max
    barrier, so x streams: DMA block -> ACT scale+round to i16 -> DVE clamp to
    integer-valued bf16/fp8 -> matmul, tile by tile.
  - s_w is computed on device: per-chunk DVE abs-max rides each w DMA piece,
    gpsimd partition_all_reduce folds partitions, reciprocal.  Quantization rounds
    via the f32->i16 convert (nearest-even, matches jnp.round); the +-127
    tensor_scalar clamp realizes the clip exactly.
  - integer matmul on the PE; fp32 PSUM accumulation is exact (|sum| < 2^24).
    The bias rides in as a K=1 *bf16* matmul (bias*s_x*s_w in bf16: ~0.4% of a term
    that is ~2% of y's rms -- negligible), so accumulation groups never see fp32.
  - FP8_CHUNKS of the 8 k-chunks are stored as fp8e4m3 and contracted with
    DoubleRow matmuls (2 MACs/cell/cycle, K=256/instruction).  e4m3 keeps only 4
    significant bits of the 8-bit integers, adding ~2.6% rms noise to those
    products; the measured end-to-end max rel err is ~2.1e-2 for all 8 chunks and
    scales as sqrt(FP8_CHUNKS/8) (~1.5e-2 at 4) vs the 2e-2 gate.
  - LayerNorm is scale-invariant, so PSUM integers are never dequantized: the
    residual is scaled by s_x*s_w inside the fused scalar_tensor_tensor epilogue
    (which also emits the row sum), ACT Square+accum gives the sum of squares,
    per-2-tile stats -> rstd, DVE tensor_scalar applies (y-mu)*rstd.
  - output is stored bf16 tile-blocked (LN output is O(1); bf16 rounding is ~1e-3
    abs), widened to f32 and unblocked on the host (exact relayout).
"""

import numpy as np

P = 128
T = 2048   # tokens per core (S of one batch element)
H = 1024   # hidden
KO = 8     # k chunks of 128 (H / P)
XBLK = 2   # t-tiles per x/res/out DMA block
GROUP = 2  # t-tiles per stats group

S_X = 127.0 / 2.5  # layerwise activation quant scale (see module docstring)

# fp8 DoubleRow measured: bass/walrus lowers one DR matmul into TWO HW matmuls
# (518ns per K=256 vs bf16's 470ns) -- no win, plus ~1.5e-2 error.  Keep 0.
FP8_CHUNKS = 0  # how many of the KO k-chunks use fp8 DoubleRow (even; 0 disables)

_CACHE = {}


def _build(trivial_affine: bool, fp8_chunks: int, t=T, h=H):
    import concourse.bass as bass
    import concourse.bass_isa as bass_isa
    import concourse.bacc as bacc
    import concourse.mybir as mybir
    import concourse.tile as tile

    ko = h // P
    f8 = fp8_chunks
    assert f8 % 2 == 0 and 0 <= f8 <= ko
    nt = t // P                  # t-tiles
    nb = nt // XBLK             # x/res/out DMA blocks
    tb = XBLK * P               # tokens per block
    group = min(GROUP, nt)
    f32 = mybir.dt.float32
    bf16 = mybir.dt.bfloat16
    i16 = mybir.dt.int16
    fp8 = mybir.dt.float8e4
    Alu = mybir.AluOpType
    Act = mybir.ActivationFunctionType
    DR = mybir.MatmulPerfMode.DoubleRow

    nc = bacc.Bacc("TRN2", target_bir_lowering=False, debug=False)

    # x: [nb, P, ko*tb] f32, tile-contiguous: x4[b, p, c*tb + i] = x.T[c*P+p, b*tb + i]
    x4 = nc.dram_tensor("x4", [nb, P, ko * tb], f32, kind="ExternalInput").ap()
    # res: [nb, P, tb/P*h] f32 tile-blocked: res_m[b, p, q*h + o] = res[b*tb + q*P + p, o]
    res = nc.dram_tensor("res", [nb, P, XBLK * h], f32, kind="ExternalInput").ap()
    # w: [P, ko*h] f32: w3[p, c*h + o] = weight[o, c*P+p]
    w3 = nc.dram_tensor("w3", [P, ko * h], f32, kind="ExternalInput").ap()
    bias_d = nc.dram_tensor("bias", [h], f32, kind="ExternalInput").ap()
    gamma_d = nc.dram_tensor("gamma", [h], f32, kind="ExternalInput").ap()
    beta_d = nc.dram_tensor("beta", [h], f32, kind="ExternalInput").ap()
    # out: [nb, P, tb/P*h] bf16 tile-blocked (host unblocks + widens)
    out_d = nc.dram_tensor("out", [nb, P, XBLK * h], bf16, kind="ExternalOutput").ap()

    with tile.TileContext(nc) as tc:
        keep = tc.alloc_tile_pool(name="keep", bufs=1)
        p1 = tc.alloc_tile_pool(name="p1", bufs=1)

        # ---- persistent tiles ----
        wq8 = keep.tile([P, f8 * h], fp8, name="wq8") if f8 else None
        wqb = keep.tile([P, (ko - f8) * h], bf16, name="wqb") if f8 < ko else None
        ones_bf = keep.tile([1, P], bf16)
        nc.vector.memset(ones_bf, 1.0)
        bias_sb = keep.tile([1, h], f32)
        nc.sync.dma_start(out=bias_sb, in_=bias_d[None, :])
        bias_bf = keep.tile([1, h], bf16)  # bias * s_x * s_w
        wmax8 = keep.tile([P, ko], f32)
        wmax_p = keep.tile([P, 1], f32)
        wmax_a = keep.tile([P, 1], f32)    # all-reduced |w| max (same value on all partitions)
        s_w = keep.tile([P, 1], f32)
        ssw = keep.tile([P, 1], f32)       # s_x * s_w (residual/bias pre-scale)
        stat_sum = keep.tile([P, nt], f32)
        stat_sq = keep.tile([P, nt], f32)
        mu = keep.tile([P, nt], f32)
        rstd = keep.tile([P, nt], f32)
        nmurs = keep.tile([P, nt], f32)    # -mu * rstd
        if not trivial_affine:
            gam_rep = keep.tile([P, h], f32)
            bet_rep = keep.tile([P, h], f32)
            nc.sync.dma_start(out=gam_rep, in_=gamma_d[None, :].to_broadcast((P, h)))
            nc.sync.dma_start(out=bet_rep, in_=beta_d[None, :].to_broadcast((P, h)))

        # ---- streaming pools (x chain 0 interleaves into the w load below) ----
        pool_xf = tc.alloc_tile_pool(name="xf", bufs=3)
        pool_xi = tc.alloc_tile_pool(name="xi", bufs=2)
        pool_xq = tc.alloc_tile_pool(name="xq", bufs=4)
        pool_rt = tc.alloc_tile_pool(name="rt", bufs=3)
        pool_yt = tc.alloc_tile_pool(name="yt", bufs=6)
        pool_sq = tc.alloc_tile_pool(name="sq", bufs=2)
        pool_ot = tc.alloc_tile_pool(name="ot", bufs=2)
        pool_ps = tc.alloc_tile_pool(name="ps", bufs=4, space="PSUM")

        xqs = {}
        xfs = {}

        def emit_xchain(b, dma=True):
            if dma:
                xf = pool_xf.tile([P, ko * tb], f32, tag="xf", name=f"xf_{b}")
                nc.sync.dma_start(out=xf, in_=x4[b])
            else:
                xf = xfs.pop(b)
            xi = pool_xi.tile([P, ko * tb], i16, tag="xi", name=f"xi_{b}")
            nc.scalar.activation(out=xi, in_=xf, func=Act.Identity, scale=S_X, bias=0.0)
            x8 = xb = None
            if f8:
                x8 = pool_xq.tile([P, f8 * tb], fp8, tag="x8", name=f"x8_{b}")
                nc.vector.tensor_scalar(
                    out=x8, in0=xi[:, : f8 * tb], scalar1=127.0, scalar2=-127.0,
                    op0=Alu.min, op1=Alu.max,
                )
            if f8 < ko:
                xb = pool_xq.tile([P, (ko - f8) * tb], bf16, tag="xb", name=f"xb_{b}")
                nc.vector.tensor_scalar(
                    out=xb, in0=xi[:, f8 * tb :], scalar1=127.0, scalar2=-127.0,
                    op0=Alu.min, op1=Alu.max,
                )
            xqs[b] = (x8, xb)

        # ---- load weight (gates the PE pipeline; x block 0 rides between w
        # chunks on the FIFO load ring so neither serializes the other fully) ----
        wf = p1.tile([P, ko * h], f32)
        for c2 in range(4):
            sl = slice(c2 * 2 * h, (c2 + 1) * 2 * h)
            nc.sync.dma_start(out=wf[:, sl], in_=w3[:, sl])
            if c2 == 2:
                # x block 0 DMA between w chunks; its quant chain is emitted
                # after the abs-max ops so it can't head-of-line block the
                # DVE FIFO that the s_w chain runs on.
                xfs[0] = pool_xf.tile([P, ko * tb], f32, tag="xf", name="xf_0")
                nc.sync.dma_start(out=xfs[0], in_=x4[0])
            for k in range(2):
                c = 2 * c2 + k
                nc.vector.tensor_reduce(
                    out=wmax8[:, c : c + 1], in_=wf[:, c * h : (c + 1) * h],
                    axis=mybir.AxisListType.X, op=Alu.max, apply_absolute_value=True,
                )
        nc.vector.tensor_reduce(
            out=wmax_p, in_=wmax8, axis=mybir.AxisListType.X, op=Alu.max,
        )
        emit_xchain(0, dma=False)
        nc.gpsimd.partition_all_reduce(
            wmax_a, wmax_p, channels=P, reduce_op=bass_isa.ReduceOp.absmax,
        )
        # m = min(max|w|, clip); the +-127 clamp after rounding realizes the clip
        nc.vector.tensor_scalar_min(out=wmax_a, in0=wmax_a, scalar1=2.5)
        nc.vector.reciprocal(out=s_w, in_=wmax_a)
        nc.vector.tensor_scalar_mul(out=s_w, in0=s_w, scalar1=127.0)
        nc.vector.tensor_scalar_mul(out=ssw, in0=s_w, scalar1=S_X)
        nc.vector.tensor_scalar_mul(out=bias_bf, in0=bias_sb, scalar1=ssw[0:1, 0:1])

        # quantize weight, all on DVE (the critical path to the first matmul):
        # round(w*s_w) via the f32->i16 convert (nearest-even, matches jnp.round,
        # HW-verified identical to the ACT convert), then the +-127 clamp during
        # the i16 -> bf16/fp8 convert.  8 chunks so wq trickles to the PE.
        def wq_dst(c):
            if c < f8:
                return wq8[:, c * h : (c + 1) * h]
            return wqb[:, (c - f8) * h : (c - f8 + 1) * h]

        for c in range(ko):
            wi16 = p1.tile([P, h], i16, tag="wi16", name=f"wi16_{c}", bufs=2)
            nc.vector.tensor_scalar_mul(
                out=wi16, in0=wf[:, c * h : (c + 1) * h], scalar1=s_w,
            )
            nc.vector.tensor_scalar(
                out=wq_dst(c), in0=wi16,
                scalar1=127.0, scalar2=-127.0, op0=Alu.min, op1=Alu.max,
            )
        wq8_v = wq8.rearrange("p (c k o) -> p c k o", c=max(f8 // 2, 1), k=2) if f8 else None

        # ---- streaming main loop ----
        half = h // 2
        emit_xchain(1)
        yts = {}
        for b in range(nb):
            x8, xb = xqs.pop(b)
            x8_v = x8.rearrange("p (c k t) -> p c k t", c=max(f8 // 2, 1), k=2) if f8 else None
            rt = pool_rt.tile([P, XBLK * h], f32, tag="rt", name=f"rt_{b}")
            nc.sync.dma_start(out=rt, in_=res[b])

            for q in range(XBLK):
                j = XBLK * b + q
                ps = pool_ps.tile([P, h], f32, tag="ps", name=f"ps_{j}")
                for nf in range(2):
                    ocol = slice(nf * half, (nf + 1) * half)
                    nc.tensor.matmul(
                        ps[:, ocol], lhsT=ones_bf, rhs=bias_bf[:, ocol],
                        start=True, stop=False,
                    )
                for c2 in range(f8 // 2):
                    lhs = x8_v[:, c2, :, q * P : (q + 1) * P]
                    last = f8 == ko and c2 == f8 // 2 - 1
                    for nf in range(2):
                        ocol = slice(nf * half, (nf + 1) * half)
                        nc.tensor.matmul(
                            ps[:, ocol], lhsT=lhs, rhs=wq8_v[:, c2, :, ocol],
                            start=False, stop=last, perf_mode=DR,
                        )
                for ci in range(ko - f8):
                    lhs = xb[:, ci * tb + q * P : ci * tb + (q + 1) * P]
                    for nf in range(2):
                        ocol = slice(nf * half, (nf + 1) * half)
                        nc.tensor.matmul(
                            ps[:, ocol], lhsT=lhs,
                            rhs=wqb[:, ci * h + nf * half : ci * h + (nf + 1) * half],
                            start=False, stop=(ci == ko - f8 - 1),
                        )
                # y = res*(s_x*s_w) + psum ; accum_out = row-sum of y
                yt = pool_yt.tile([P, h], f32, tag="yt", name=f"yt_{j}")
                yts[j] = yt
                nc.vector.scalar_tensor_tensor(
                    out=yt, in0=rt[:, q * h : (q + 1) * h], scalar=ssw, in1=ps,
                    op0=Alu.mult, op1=Alu.add,
                    accum_out=stat_sum[:, j : j + 1],
                )
                # sum of squares on ACT (output tensor is a throwaway)
                sq = pool_sq.tile([P, h], bf16, tag="sq", name=f"sq_{j}")
                nc.scalar.activation(
                    out=sq, in_=yt, func=Act.Square,
                    accum_out=stat_sq[:, j : j + 1],
                )

            if b + 2 < nb:
                emit_xchain(b + 2)

            if (b + 1) * XBLK % group == 0:
                # ---- batched stats for the group ----
                g0 = (b + 1) * XBLK - group
                gsl = slice(g0, g0 + group)
                musl = mu[:, gsl]
                nc.vector.tensor_scalar_mul(out=musl, in0=stat_sum[:, gsl], scalar1=1.0 / h)
                var = rstd[:, gsl]  # slot reused: var -> sd -> rstd
                nc.vector.tensor_scalar_mul(out=var, in0=stat_sq[:, gsl], scalar1=1.0 / h)
                mu2 = pool_sq.tile([P, group], f32, tag="mu2", name=f"mu2_{g0}")
                nc.vector.tensor_tensor(mu2, musl, musl, Alu.mult)
                nc.vector.tensor_tensor(var, var, mu2, Alu.subtract)
                nc.scalar.sqrt(out=var, in_=var)
                nc.vector.reciprocal(out=var, in_=var)
                nc.vector.tensor_tensor(nmurs[:, gsl], musl, var, Alu.mult)
                nc.vector.tensor_scalar_mul(out=nmurs[:, gsl], in0=nmurs[:, gsl], scalar1=-1.0)
                # ---- normalize + store (per DMA block; stores ride the scalar
                # HWDGE ring so they never stall the x/res load ring) ----
                for b2 in range(g0 // XBLK, (g0 + group) // XBLK):
                    ot = pool_ot.tile([P, XBLK * h], bf16, tag="ot", name=f"ot_{b2}")
                    for q in range(XBLK):
                        j = XBLK * b2 + q
                        yt = yts.pop(j)
                        osl = slice(q * h, (q + 1) * h)
                        # (y - mu) * rstd on DVE (2x single-tensor mode), bf16 out
                        nc.vector.tensor_scalar(
                            out=ot[:, osl], in0=yt,
                            scalar1=rstd[:, j : j + 1], scalar2=nmurs[:, j : j + 1],
                            op0=Alu.mult, op1=Alu.add,
                        )
                        if not trivial_affine:
                            nc.vector.tensor_tensor(ot[:, osl], ot[:, osl], gam_rep, Alu.mult)
                            nc.vector.tensor_tensor(ot[:, osl], ot[:, osl], bet_rep, Alu.add)
                    nc.scalar.dma_start(out=out_d[b2], in_=ot)

        for p in (pool_ps, pool_ot, pool_sq, pool_yt, pool_rt, pool_xq, pool_xi, pool_xf, p1, keep):
            p.release()

    if not nc.is_finalized():
        nc.finalize()
    return nc


def _get_nc(trivial_affine: bool, t=T, h=H):
    key = (trivial_affine, FP8_CHUNKS, t, h)
    if key not in _CACHE:
        _CACHE[key] = _build(trivial_affine, FP8_CHUNKS, t, h)
    return _CACHE[key]


def make_in_maps(hidden_states, input_tensor, weight, bias, gamma, beta):
    """Host-side marshalling: pure relayout except the (practically impossible)
    no-clip fallback, where an exact prescale keeps the device math identical to
    the reference (see module docstring)."""
    hidden_states = np.asarray(hidden_states, dtype=np.float32)
    input_tensor = np.asarray(input_tensor, dtype=np.float32)
    weight = np.asarray(weight, dtype=np.float32)
    bias = np.asarray(bias, dtype=np.float32)
    gamma = np.asarray(gamma, dtype=np.float32)
    beta = np.asarray(beta, dtype=np.float32)

    B, S, HH = hidden_states.shape
    ko = HH // P
    nb = S // P // XBLK
    tb = XBLK * P

    m = float(np.abs(hidden_states).max())
    if m < 2.5:
        # reference scale would be 127/m; prescaling x/res/bias by 2.5/m makes
        # round(x'*S_X) the exact reference integers and S_X*res' the exact
        # reference residual scaling (LN is scale-invariant).
        f = 2.5 / m
        hidden_states = hidden_states * f
        input_tensor = input_tensor * f
        bias = bias * f

    w3 = np.ascontiguousarray(
        weight.T.reshape(ko, P, HH).transpose(1, 0, 2)
    ).reshape(P, ko * HH)
    in_maps = []
    for c in range(B):
        x4 = np.ascontiguousarray(
            hidden_states[c].reshape(nb, tb, ko, P).transpose(0, 3, 2, 1)
        ).reshape(nb, P, ko * tb)
        res_m = np.ascontiguousarray(
            input_tensor[c].reshape(nb, XBLK, P, HH).transpose(0, 2, 1, 3)
        ).reshape(nb, P, XBLK * HH)
        in_maps.append(
            {
                "x4": x4,
                "res": res_m,
                "w3": w3,
                "bias": bias,
                "gamma": gamma,
                "beta": beta,
            }
        )
    return in_maps


def gather_out(results, B, S=T, HH=H):
    """Unblock the tiled bf16 output and widen to f32 (exact relayout)."""
    nb = S // P // XBLK
    outs = []
    for c in range(B):
        o = np.asarray(results[c]["out"]).astype(np.float32)
        outs.append(o.reshape(nb, P, XBLK, HH).transpose(0, 2, 1, 3).reshape(S, HH))
    return np.stack(outs)


def kernel(hidden_states, input_tensor, weight, bias, gamma, beta):
    from concourse.bass_utils import run_bass_kernel_spmd

    gamma = np.asarray(gamma, dtype=np.float32)
    beta = np.asarray(beta, dtype=np.float32)
    B, S, HH = np.asarray(hidden_states).shape
    trivial = bool(np.all(gamma == 1.0) and np.all(beta == 0.0))
    nc = _get_nc(trivial, S, HH)

    in_maps = make_in_maps(hidden_states, input_tensor, weight, bias, gamma, beta)
    r = run_bass_kernel_spmd(nc, in_maps, core_ids=list(range(B)))
    return gather_out(r.results, B, S, HH)


# revision 18
# speedup vs baseline: 1.2025x; 1.0033x over previous
"""Trainium2 Bass kernel for nn_BertSelfOutput (BiT 8-bit quantized BertSelfOutput).

Computation (see reference):
    wq = sym_quant(weight, clip=2.5, bits=8)       # layerwise scale s_w = 127/max|clip(w)|
    xq = sym_quant(hidden_states, clip=2.5, bits=8)
    h  = xq @ wq.T + bias
    y  = LayerNorm(h + input_tensor) * gamma + beta

Sharding: data-parallel over batch (8 cores, 1 batch element each); weight/bias/LN
params replicated.  Host-side marshalling is pure relayout (transpose/reshape): x,
res and the weight are laid out so every DMA is contiguous per SBUF partition
(8KB+ descriptors, near line rate), with the contraction dim on partitions.

Device algorithm per core (streaming, DMA/PE co-limited):
  - s_x is a compile-time constant 127/2.5: the layerwise clip at 2.5 makes
    max|clip(x)| == 2.5 whenever any element of the 2M-sample N(0,1) shard clips,
    which is a certainty at this size (kernel() enforces it with an exact host-side
    prescale fallback for the impossible case).  This removes the global abs-max
    barrier, so x streams: DMA block -> ACT scale+round to i16 -> DVE clamp to
    integer-valued bf16/fp8 -> matmul, tile by tile.
  - s_w is computed on device: per-chunk DVE abs-max rides each w DMA piece,
    gpsimd partition_all_reduce folds partitions, reciprocal.  Quantization rounds
    via the f32->i16 convert (nearest-even, matches jnp.round); the +-127
    tensor_scalar clamp realizes the clip exactly.
  - integer matmul on the PE; fp32 PSUM accumulation is exact (|sum| < 2^24).
    The bias rides in as a K=1 *bf16* matmul (bias*s_x*s_w in bf16: ~0.4% of a term
    that is ~2% of y's rms -- negligible), so accumulation groups never see fp32.
  - FP8_CHUNKS of the 8 k-chunks are stored as fp8e4m3 and contracted with
    DoubleRow matmuls (2 MACs/cell/cycle, K=256/instruction).  e4m3 keeps only 4
    significant bits of the 8-bit integers, adding ~2.6% rms noise to those
    products; the measured end-to-end max rel err is ~2.1e-2 for all 8 chunks and
    scales as sqrt(FP8_CHUNKS/8) (~1.5e-2 at 4) vs the 2e-2 gate.
  - LayerNorm is scale-invariant, so PSUM integers are never dequantized: the
    residual is scaled by s_x*s_w inside the fused scalar_tensor_tensor epilogue
    (which also emits the row sum), ACT Square+accum gives the sum of squares,
    per-2-tile stats -> rstd, DVE tensor_scalar applies (y-mu)*rstd.
  - output is stored bf16 tile-blocked (LN output is O(1); bf16 rounding is ~1e-3
    abs), widened to f32 and unblocked on the host (exact relayout).
"""

import numpy as np

P = 128
T = 2048   # tokens per core (S of one batch element)
H = 1024   # hidden
KO = 8     # k chunks of 128 (H / P)
XBLK = 2   # t-tiles per x/res/out DMA block
GROUP = 2  # t-tiles per stats group

S_X = 127.0 / 2.5  # layerwise activation quant scale (see module docstring)

# fp8 DoubleRow measured: bass/walrus lowers one DR matmul into TWO HW matmuls
# (518ns per K=256 vs bf16's 470ns) -- no win, plus ~1.5e-2 error.  Keep 0.
FP8_CHUNKS = 0  # how many of the KO k-chunks use fp8 DoubleRow (even; 0 disables)

_CACHE = {}


def _build(trivial_affine: bool, fp8_chunks: int, t=T, h=H):
    import concourse.bass as bass
    import concourse.bass_isa as bass_isa
    import concourse.bacc as bacc
    import concourse.mybir as mybir
    import concourse.tile as tile

    ko = h // P
    f8 = fp8_chunks
    assert f8 % 2 == 0 and 0 <= f8 <= ko
    nt = t // P                  # t-tiles
    nb = nt // XBLK             # x/res/out DMA blocks
    tb = XBLK * P               # tokens per block
    group = min(GROUP, nt)
    f32 = mybir.dt.float32
    bf16 = mybir.dt.bfloat16
    i16 = mybir.dt.int16
    fp8 = mybir.dt.float8e4
    Alu = mybir.AluOpType
    Act = mybir.ActivationFunctionType
    DR = mybir.MatmulPerfMode.DoubleRow

    nc = bacc.Bacc("TRN2", target_bir_lowering=False, debug=False)

    # x: [nb, P, ko*tb] f32, tile-contiguous: x4[b, p, c*tb + i] = x.T[c*P+p, b*tb + i]
    x4 = nc.dram_tensor("x4", [nb, P, ko * tb], f32, kind="ExternalInput").ap()
    # res: [nb, P, tb/P*h] f32 tile-blocked: res_m[b, p, q*h + o] = res[b*tb + q*P + p, o]
    res = nc.dram_tensor("res", [nb, P, XBLK * h], f32, kind="ExternalInput").ap()
    # w: [P, ko*h] f32: w3[p, c*h + o] = weight[o, c*P+p]
    w3 = nc.dram_tensor("w3", [P, ko * h], f32, kind="ExternalInput").ap()
    bias_d = nc.dram_tensor("bias", [h], f32, kind="ExternalInput").ap()
    gamma_d = nc.dram_tensor("gamma", [h], f32, kind="ExternalInput").ap()
    beta_d = nc.dram_tensor("beta", [h], f32, kind="ExternalInput").ap()
    # out: [nb, P, tb/P*h] bf16 tile-blocked (host unblocks + widens)
    out_d = nc.dram_tensor("out", [nb, P, XBLK * h], bf16, kind="ExternalOutput").ap()

    with tile.TileContext(nc) as tc:
        keep = tc.alloc_tile_pool(name="keep", bufs=1)
        p1 = tc.alloc_tile_pool(name="p1", bufs=1)

        # ---- persistent tiles ----
        wq8 = keep.tile([P, f8 * h], fp8, name="wq8") if f8 else None
        wqb = keep.tile([P, (ko - f8) * h], bf16, name="wqb") if f8 < ko else None
        ones_bf = keep.tile([1, P], bf16)
        nc.vector.memset(ones_bf, 1.0)
        bias_sb = keep.tile([1, h], f32)
        nc.sync.dma_start(out=bias_sb, in_=bias_d[None, :])
        bias_bf = keep.tile([1, h], bf16)  # bias * s_x * s_w
        wmax8 = keep.tile([P, ko], f32)
        wmax_p = keep.tile([P, 1], f32)
        wmax_a = keep.tile([P, 1], f32)    # all-reduced |w| max (same value on all partitions)
        s_w = keep.tile([P, 1], f32)
        ssw = keep.tile([P, 1], f32)       # s_x * s_w (residual/bias pre-scale)
        stat_sum = keep.tile([P, nt], f32)
        stat_sq = keep.tile([P, nt], f32)
        mu = keep.tile([P, nt], f32)
        rstd = keep.tile([P, nt], f32)
        nmurs = keep.tile([P, nt], f32)    # -mu * rstd
        if not trivial_affine:
            gam_rep = keep.tile([P, h], f32)
            bet_rep = keep.tile([P, h], f32)
            nc.sync.dma_start(out=gam_rep, in_=gamma_d[None, :].to_broadcast((P, h)))
            nc.sync.dma_start(out=bet_rep, in_=beta_d[None, :].to_broadcast((P, h)))

        # ---- streaming pools (x chain 0 interleaves into the w load below) ----
        pool_xf = tc.alloc_tile_pool(name="xf", bufs=3)
        pool_xi = tc.alloc_tile_pool(name="xi", bufs=2)
        pool_xq = tc.alloc_tile_pool(name="xq", bufs=4)
        pool_rt = tc.alloc_tile_pool(name="rt", bufs=3)
        pool_yt = tc.alloc_tile_pool(name="yt", bufs=6)
        pool_sq = tc.alloc_tile_pool(name="sq", bufs=2)
        pool_ot = tc.alloc_tile_pool(name="ot", bufs=2)
        pool_ps = tc.alloc_tile_pool(name="ps", bufs=4, space="PSUM")

        xqs = {}
        xfs = {}

        def emit_xchain(b, dma=True):
            if dma:
                xf = pool_xf.tile([P, ko * tb], f32, tag="xf", name=f"xf_{b}")
                nc.sync.dma_start(out=xf, in_=x4[b])
            else:
                xf = xfs.pop(b)
            xi = pool_xi.tile([P, ko * tb], i16, tag="xi", name=f"xi_{b}")
            nc.scalar.activation(out=xi, in_=xf, func=Act.Identity, scale=S_X, bias=0.0)
            x8 = xb = None
            if f8:
                x8 = pool_xq.tile([P, f8 * tb], fp8, tag="x8", name=f"x8_{b}")
                nc.vector.tensor_scalar(
                    out=x8, in0=xi[:, : f8 * tb], scalar1=127.0, scalar2=-127.0,
                    op0=Alu.min, op1=Alu.max,
                )
            if f8 < ko:
                xb = pool_xq.tile([P, (ko - f8) * tb], bf16, tag="xb", name=f"xb_{b}")
                nc.vector.tensor_scalar(
                    out=xb, in0=xi[:, f8 * tb :], scalar1=127.0, scalar2=-127.0,
                    op0=Alu.min, op1=Alu.max,
                )
            xqs[b] = (x8, xb)

        # ---- load weight (gates the PE pipeline; x block 0 rides between w
        # chunks on the FIFO load ring so neither serializes the other fully) ----
        wf = p1.tile([P, ko * h], f32)
        # last chunk is a single [P, h] piece so the abs-max tail on the s_w
        # critical path is one 1us reduce, not two
        wchunks = [(0, 2), (2, 4), (4, 6), (6, 7), (7, 8)]
        for i, (c0, c1) in enumerate(wchunks):
            nc.sync.dma_start(out=wf[:, c0 * h : c1 * h], in_=w3[:, c0 * h : c1 * h])
            if i == 2:
                # x block 0 DMA between w chunks; its quant chain is emitted
                # after the abs-max ops so it can't head-of-line block the
                # DVE FIFO that the s_w chain runs on.
                xfs[0] = pool_xf.tile([P, ko * tb], f32, tag="xf", name="xf_0")
                nc.sync.dma_start(out=xfs[0], in_=x4[0])
            for c in range(c0, c1):
                nc.vector.tensor_reduce(
                    out=wmax8[:, c : c + 1], in_=wf[:, c * h : (c + 1) * h],
                    axis=mybir.AxisListType.X, op=Alu.max, apply_absolute_value=True,
                )
        nc.vector.tensor_reduce(
            out=wmax_p, in_=wmax8, axis=mybir.AxisListType.X, op=Alu.max,
        )
        emit_xchain(0, dma=False)
        nc.gpsimd.partition_all_reduce(
            wmax_a, wmax_p, channels=P, reduce_op=bass_isa.ReduceOp.absmax,
        )
        # m = min(max|w|, clip); the +-127 clamp after rounding realizes the clip
        nc.vector.tensor_scalar_min(out=wmax_a, in0=wmax_a, scalar1=2.5)
        nc.vector.reciprocal(out=s_w, in_=wmax_a)
        nc.vector.tensor_scalar_mul(out=s_w, in0=s_w, scalar1=127.0)
        nc.vector.tensor_scalar_mul(out=ssw, in0=s_w, scalar1=S_X)
        nc.vector.tensor_scalar_mul(out=bias_bf, in0=bias_sb, scalar1=ssw[0:1, 0:1])

        # quantize weight, all on DVE (the critical path to the first matmul):
        # round(w*s_w) via the f32->i16 convert (nearest-even, matches jnp.round,
        # HW-verified identical to the ACT convert), then the +-127 clamp during
        # the i16 -> bf16/fp8 convert.  8 chunks so wq trickles to the PE.
        def wq_dst(c):
            if c < f8:
                return wq8[:, c * h : (c + 1) * h]
            return wqb[:, (c - f8) * h : (c - f8 + 1) * h]

        for c in range(ko):
            wi16 = p1.tile([P, h], i16, tag="wi16", name=f"wi16_{c}", bufs=2)
            nc.vector.tensor_scalar_mul(
                out=wi16, in0=wf[:, c * h : (c + 1) * h], scalar1=s_w,
            )
            nc.vector.tensor_scalar(
                out=wq_dst(c), in0=wi16,
                scalar1=127.0, scalar2=-127.0, op0=Alu.min, op1=Alu.max,
            )
        wq8_v = wq8.rearrange("p (c k o) -> p c k o", c=max(f8 // 2, 1), k=2) if f8 else None

        # ---- streaming main loop ----
        half = h // 2
        emit_xchain(1)
        yts = {}
        for b in range(nb):
            x8, xb = xqs.pop(b)
            x8_v = x8.rearrange("p (c k t) -> p c k t", c=max(f8 // 2, 1), k=2) if f8 else None
            rt = pool_rt.tile([P, XBLK * h], f32, tag="rt", name=f"rt_{b}")
            nc.sync.dma_start(out=rt, in_=res[b])

            for q in range(XBLK):
                j = XBLK * b + q
                ps = pool_ps.tile([P, h], f32, tag="ps", name=f"ps_{j}")
                for nf in range(2):
                    ocol = slice(nf * half, (nf + 1) * half)
                    nc.tensor.matmul(
                        ps[:, ocol], lhsT=ones_bf, rhs=bias_bf[:, ocol],
                        start=True, stop=False,
                    )
                for c2 in range(f8 // 2):
                    lhs = x8_v[:, c2, :, q * P : (q + 1) * P]
                    last = f8 == ko and c2 == f8 // 2 - 1
                    for nf in range(2):
                        ocol = slice(nf * half, (nf + 1) * half)
                        nc.tensor.matmul(
                            ps[:, ocol], lhsT=lhs, rhs=wq8_v[:, c2, :, ocol],
                            start=False, stop=last, perf_mode=DR,
                        )
                for ci in range(ko - f8):
                    lhs = xb[:, ci * tb + q * P : ci * tb + (q + 1) * P]
                    for nf in range(2):
                        ocol = slice(nf * half, (nf + 1) * half)
                        nc.tensor.matmul(
                            ps[:, ocol], lhsT=lhs,
                            rhs=wqb[:, ci * h + nf * half : ci * h + (nf + 1) * half],
                            start=False, stop=(ci == ko - f8 - 1),
                        )
                # y = res*(s_x*s_w) + psum ; accum_out = row-sum of y
                yt = pool_yt.tile([P, h], f32, tag="yt", name=f"yt_{j}")
                yts[j] = yt
                nc.vector.scalar_tensor_tensor(
                    out=yt, in0=rt[:, q * h : (q + 1) * h], scalar=ssw, in1=ps,
                    op0=Alu.mult, op1=Alu.add,
                    accum_out=stat_sum[:, j : j + 1],
                )
                # sum of squares on ACT (output tensor is a throwaway)
                sq = pool_sq.tile([P, h], bf16, tag="sq", name=f"sq_{j}")
                nc.scalar.activation(
                    out=sq, in_=yt, func=Act.Square,
                    accum_out=stat_sq[:, j : j + 1],
                )

            if b + 2 < nb:
                emit_xchain(b + 2)

            if (b + 1) * XBLK % group == 0:
                # ---- batched stats for the group ----
                g0 = (b + 1) * XBLK - group
                gsl = slice(g0, g0 + group)
                musl = mu[:, gsl]
                nc.vector.tensor_scalar_mul(out=musl, in0=stat_sum[:, gsl], scalar1=1.0 / h)
                var = rstd[:, gsl]  # slot reused: var -> sd -> rstd
                nc.vector.tensor_scalar_mul(out=var, in0=stat_sq[:, gsl], scalar1=1.0 / h)
                mu2 = pool_sq.tile([P, group], f32, tag="mu2", name=f"mu2_{g0}")
                nc.vector.tensor_tensor(mu2, musl, musl, Alu.mult)
                nc.vector.tensor_tensor(var, var, mu2, Alu.subtract)
                nc.scalar.sqrt(out=var, in_=var)
                nc.vector.reciprocal(out=var, in_=var)
                nc.vector.tensor_tensor(nmurs[:, gsl], musl, var, Alu.mult)
                nc.vector.tensor_scalar_mul(out=nmurs[:, gsl], in0=nmurs[:, gsl], scalar1=-1.0)
                # ---- normalize + store (per DMA block; stores ride the scalar
                # HWDGE ring so they never stall the x/res load ring) ----
                for b2 in range(g0 // XBLK, (g0 + group) // XBLK):
                    ot = pool_ot.tile([P, XBLK * h], bf16, tag="ot", name=f"ot_{b2}")
                    for q in range(XBLK):
                        j = XBLK * b2 + q
                        yt = yts.pop(j)
                        osl = slice(q * h, (q + 1) * h)
                        # (y - mu) * rstd on DVE (2x single-tensor mode), bf16 out
                        nc.vector.tensor_scalar(
                            out=ot[:, osl], in0=yt,
                            scalar1=rstd[:, j : j + 1], scalar2=nmurs[:, j : j + 1],
                            op0=Alu.mult, op1=Alu.add,
                        )
                        if not trivial_affine:
                            nc.vector.tensor_tensor(ot[:, osl], ot[:, osl], gam_rep, Alu.mult)
                            nc.vector.tensor_tensor(ot[:, osl], ot[:, osl], bet_rep, Alu.add)
                    # per-tile store halves: the first half dispatches without
                    # waiting the second tile's normalize (shorter tail)
                    for q in range(XBLK):
                        osl = slice(q * h, (q + 1) * h)
                        nc.scalar.dma_start(out=out_d[b2][:, osl], in_=ot[:, osl])

        for p in (pool_ps, pool_ot, pool_sq, pool_yt, pool_rt, pool_xq, pool_xi, pool_xf, p1, keep):
            p.release()

    if not nc.is_finalized():
        nc.finalize()
    return nc


def _get_nc(trivial_affine: bool, t=T, h=H):
    key = (trivial_affine, FP8_CHUNKS, t, h)
    if key not in _CACHE:
        _CACHE[key] = _build(trivial_affine, FP8_CHUNKS, t, h)
    return _CACHE[key]


def make_in_maps(hidden_states, input_tensor, weight, bias, gamma, beta):
    """Host-side marshalling: pure relayout except the (practically impossible)
    no-clip fallback, where an exact prescale keeps the device math identical to
    the reference (see module docstring)."""
    hidden_states = np.asarray(hidden_states, dtype=np.float32)
    input_tensor = np.asarray(input_tensor, dtype=np.float32)
    weight = np.asarray(weight, dtype=np.float32)
    bias = np.asarray(bias, dtype=np.float32)
    gamma = np.asarray(gamma, dtype=np.float32)
    beta = np.asarray(beta, dtype=np.float32)

    B, S, HH = hidden_states.shape
    ko = HH // P
    nb = S // P // XBLK
    tb = XBLK * P

    m = float(np.abs(hidden_states).max())
    if m < 2.5:
        # reference scale would be 127/m; prescaling x/res/bias by 2.5/m makes
        # round(x'*S_X) the exact reference integers and S_X*res' the exact
        # reference residual scaling (LN is scale-invariant).
        f = 2.5 / m
        hidden_states = hidden_states * f
        input_tensor = input_tensor * f
        bias = bias * f

    w3 = np.ascontiguousarray(
        weight.T.reshape(ko, P, HH).transpose(1, 0, 2)
    ).reshape(P, ko * HH)
    in_maps = []
    for c in range(B):
        x4 = np.ascontiguousarray(
            hidden_states[c].reshape(nb, tb, ko, P).transpose(0, 3, 2, 1)
        ).reshape(nb, P, ko * tb)
        res_m = np.ascontiguousarray(
            input_tensor[c].reshape(nb, XBLK, P, HH).transpose(0, 2, 1, 3)
        ).reshape(nb, P, XBLK * HH)
        in_maps.append(
            {
                "x4": x4,
                "res": res_m,
                "w3": w3,
                "bias": bias,
                "gamma": gamma,
                "beta": beta,
            }
        )
    return in_maps


def gather_out(results, B, S=T, HH=H):
    """Unblock the tiled bf16 output and widen to f32 (exact relayout)."""
    nb = S // P // XBLK
    outs = []
    for c in range(B):
        o = np.asarray(results[c]["out"]).astype(np.float32)
        outs.append(o.reshape(nb, P, XBLK, HH).transpose(0, 2, 1, 3).reshape(S, HH))
    return np.stack(outs)


def kernel(hidden_states, input_tensor, weight, bias, gamma, beta):
    from concourse.bass_utils import run_bass_kernel_spmd

    gamma = np.asarray(gamma, dtype=np.float32)
    beta = np.asarray(beta, dtype=np.float32)
    B, S, HH = np.asarray(hidden_states).shape
    trivial = bool(np.all(gamma == 1.0) and np.all(beta == 0.0))
    nc = _get_nc(trivial, S, HH)

    in_maps = make_in_maps(hidden_states, input_tensor, weight, bias, gamma, beta)
    r = run_bass_kernel_spmd(nc, in_maps, core_ids=list(range(B)))
    return gather_out(r.results, B, S, HH)
